# revision 72
# baseline (speedup 1.0000x reference)
"""3-layer GCN (gnn_message_passing) on 8 Trainium2 NeuronCores.

Sharding: nodes partitioned by range across 8 cores (dst-sharded).
Per layer, per core:
  1. z = h_shard @ W  (PE transpose per 128-node block + matmul),
     y = dinv * z  cast to bf16 (the "message table" values)
  2. Two AllGathers (first/second half of each rank's shard) -> two
     bf16 tables in DRAM; AG_a overlaps the second half of the z phase,
     AG_b overlaps phase-A gathers/matmuls.
  3. dma_gather the table rows for this core's in-edges, 2048 idxs per
     instruction spread over 4 SWDGE queues.
  4. segment-sum per 128-edge chunk via PE matmul with HOST-PRECOMPUTED
     one-hot S[edge, dst_local] tiles streamed from DRAM (S is graph
     structure, shared by all 3 layers; streaming it costs idle HBM BW
     instead of 1.3us/chunk of DVE time).
  5. epilogue per block: relu(dinv*(A@y + y)) in one ACT op, LayerNorm
     with batched reciprocal.
Self-loops are folded in via the "+ y" term (norm factorizes as
dinv[src]*dinv[dst]).
"""

import numpy as np
from contextlib import ExitStack

P = 128
D = 128          # feature width of layers (W3 zero-padded 64 -> 128)
D_OUT = 64
GQ = 8           # chunks per gather instruction (8*128 = 1024 idxs)
SGQ = 16         # chunks per one-hot S DMA load
SINGLE_PACKET = True   # 64 descs/engine, all 16 engines in parallel
NQ = 4           # SWDGE queues used round-robin


# ----------------------------------------------------------------------------
# Host-side graph preprocessing
# ----------------------------------------------------------------------------

FILL_MIN = 0.5   # identity rounds continue while >= this dst coverage


def preprocess(edge_index, n_nodes, n_cores, n_blocks, blk_a, gq=GQ):
    """Build per-core gather/scatter index arrays + one-hot S tiles.

    Nodes are split into half-shards per rank: local rows [0, blk_a*128)
    go to table_a (AllGather #1), the rest to table_b.  Table rows are
    partition-major within a rank (node (b,p) -> row p*nblk+b) so the
    y_own DRAM write is one contiguous run per SBUF partition.

    Per (block, half), edges are organized into "identity rounds": round
    k holds the k-th in-edge of each of the 128 dsts at slot=dst, so the
    scatter matmul uses a constant identity lhsT (no one-hot tile); holes
    gather a guaranteed-zero table row.  Rounds continue while dst
    coverage >= FILL_MIN; the degree tail goes to classic one-hot chunks.
    """
    npc = n_blocks * P
    split = blk_a * P                       # local row where half b starts
    rows_b = npc - split
    rows_a_pr = split + 1                   # +1 appended zero row per rank
    src = np.asarray(edge_index[0], dtype=np.int64)
    dst = np.asarray(edge_index[1], dtype=np.int64)

    deg = np.bincount(dst, minlength=n_nodes).astype(np.float32) + 1.0
    dinv = np.zeros(npc * n_cores, np.float32)
    dinv[:n_nodes] = 1.0 / np.sqrt(deg)

    r = src // npc
    off = src % npc
    bb = off // P
    pp = off % P
    in_a = off < split
    trow = np.where(in_a,
                    r * rows_a_pr + pp * blk_a + bb,
                    r * rows_b + pp * (n_blocks - blk_a) + (bb - blk_a))
    ZR_A = rows_a_pr - 1                    # rank 0's appended zero row
    ZR_B = n_cores * rows_b - 1             # padded node 50175 -> y == 0

    core_of = dst // npc
    per_core = []
    for c in range(n_cores):
        m = core_of == c
        s_t = trow[m]
        s_a = in_a[m]
        dl = dst[m] - c * npc
        blk = dl // P
        din = dl % P
        order = np.lexsort((s_t, din, ~s_a, blk))
        s_t, s_a, blk, din = s_t[order], s_a[order], blk[order], din[order]
        per_core.append((s_t, s_a, blk, din))

    def halfedges(c, b, half_a):
        s_t, s_a, blk, din = per_core[c]
        m = (blk == b) & (s_a == half_a)
        return s_t[m], din[m]   # sorted by (din, trow)

    # decide rounds (IA/IB) and tail chunk counts (SA/SB) per block,
    # shared across cores (SPMD): rounds = min over cores of the per-core
    # round count at FILL_MIN coverage; tails padded to the max.
    IA = np.zeros(n_blocks, np.int64)
    SA = np.zeros(n_blocks, np.int64)
    IB = np.zeros(n_blocks, np.int64)
    SB = np.zeros(n_blocks, np.int64)
    degs = {}
    for b in range(n_blocks):
        for half_a, I, S in ((True, IA, SA), (False, IB, SB)):
            ks = []
            for c in range(n_cores):
                t, d = halfedges(c, b, half_a)
                dg = np.bincount(d, minlength=P)
                degs[(c, b, half_a)] = dg
                k = 0
                while (dg > k).sum() >= FILL_MIN * P:
                    k += 1
                ks.append(k)
            I[b] = min(ks)
            tail_max = 0
            for c in range(n_cores):
                dg = degs[(c, b, half_a)]
                tail_max = max(tail_max, int(np.maximum(dg - I[b], 0).sum()))
            S[b] = (tail_max + P - 1) // P

    ca = IA + SA
    cb = IB + SB
    aoff = np.concatenate([[0], np.cumsum(ca)])
    boff = np.concatenate([[0], np.cumsum(cb)])
    saoff = np.concatenate([[0], np.cumsum(SA)])
    sboff = np.concatenate([[0], np.cumsum(SB)])
    nch_a, nch_b = int(aoff[-1]), int(boff[-1])
    nsa, nsb = int(saoff[-1]), int(sboff[-1])
    out = {"CA": tuple(zip(IA.tolist(), SA.tolist())),
           "CB": tuple(zip(IB.tolist(), SB.tolist())),
           "dinv": dinv, "cores": []}
    eye = np.eye(P, dtype=np.float32)
    zrow = np.zeros((P,), np.float32)
    eye_pad = np.concatenate([eye, zrow[None, :]], axis=0)  # row P = zeros
    for c in range(n_cores):
        gidx_a = np.full((nch_a, P), ZR_A, np.int64)
        gidx_b = np.full((nch_b, P), ZR_B, np.int64)
        dstid_a = np.full((nsa, P), P, np.int64)   # P -> zero row of S
        dstid_b = np.full((nsb, P), P, np.int64)

        def fill_half(b, half_a, gidx, goff, I, dstid, soff):
            t, d = halfedges(c, b, half_a)
            dg = degs[(c, b, half_a)]
            # per-dst edge lists: edges are sorted by (din, trow)
            starts = np.concatenate([[0], np.cumsum(dg)])
            K = int(I[b])
            for k in range(K):
                has = dg > k
                js = np.nonzero(has)[0]
                gidx[goff[b] + k, js] = t[starts[js] + k]
            # tail: edges with k >= K, in dst order
            tt, dd = [], []
            for j in np.nonzero(dg > K)[0]:
                tt.append(t[starts[j] + K:starts[j + 1]])
                dd.append(np.full(int(dg[j] - K), j, np.int64))
            if tt:
                tt = np.concatenate(tt)
                dd = np.concatenate(dd)
                n0 = goff[b] + K
                gidx[n0:n0 + (len(tt) + P - 1) // P].reshape(-1)[
                    :len(tt)] = tt
                dstid[soff[b]:soff[b + 1]].reshape(-1)[:len(dd)] = dd

        for b in range(n_blocks):
            fill_half(b, True, gidx_a, aoff, IA, dstid_a, saoff)
            fill_half(b, False, gidx_b, boff, IB, dstid_b, sboff)

        def wrap(flat):
            # flat [chunks, 128]; groups of `gq` chunks per gather instr;
            # within an instr: idx i -> [i % 16, i // 16], replicated 8x.
            cols = []
            for g0 in range(0, flat.shape[0], gq):
                fg = flat[g0:g0 + gq].reshape(-1)
                w16 = fg.reshape(-1, 16).T
                cols.append(np.tile(w16, (8, 1)))
            return np.ascontiguousarray(
                np.concatenate(cols, axis=1).astype(np.int16))

        def onehot(dstid):
            # dstid [chunks, 128] -> S [128 edge, chunks*128 dst] bf16
            import ml_dtypes
            s = eye_pad[dstid]                     # [chunks, 128e, 128d]
            s = s.transpose(1, 0, 2).reshape(P, -1)
            return np.ascontiguousarray(s.astype(ml_dtypes.bfloat16))

        out["cores"].append({
            "ga": wrap(gidx_a),
            "gb": wrap(gidx_b),
            "sa": onehot(dstid_a),
            "sb": onehot(dstid_b),
            "dinvb": np.ascontiguousarray(
                dinv[c * npc:(c + 1) * npc].reshape(n_blocks, P).T),
        })
    return out


def shard_x(x, n_nodes, n_cores, n_blocks):
    """x [n,128] f32 -> per-core feature-major [128 feat, n_blocks*128].

    Per block b, columns b*128..(b+1)*128 hold x[block_nodes].T so the
    layer-0 matmul can use it as lhsT directly (no PE transpose).
    """
    npc = n_blocks * P
    xp = np.zeros((npc * n_cores, x.shape[1]), np.float32)
    xp[:n_nodes] = x
    shards = []
    for c in range(n_cores):
        xs = xp[c * npc:(c + 1) * npc].reshape(n_blocks, P, x.shape[1])
        shards.append(np.ascontiguousarray(
            xs.transpose(2, 0, 1).reshape(x.shape[1], n_blocks * P)))
    return shards


# ----------------------------------------------------------------------------
# Kernel builder
# ----------------------------------------------------------------------------

def build_kernel(n_cores, n_blocks, blk_a, ca, cb, flags, eps=1e-5,
                 n_layers=3):
    """flags: per-layer tuple of (has_bias, has_g, has_be)."""
    import concourse.bacc as bacc
    import concourse.mybir as mybir
    import concourse.tile as tile
    from concourse.masks import make_identity

    f32 = mybir.dt.float32
    bf16 = mybir.dt.bfloat16
    i16 = mybir.dt.int16
    Act = mybir.ActivationFunctionType
    Alu = mybir.AluOpType

    npc = n_blocks * P
    split = blk_a * P
    rows_b = npc - split
    rows_a_pr = split + 1
    ia = [x[0] for x in ca]
    sa = [x[1] for x in ca]
    ib = [x[0] for x in cb]
    sb = [x[1] for x in cb]
    aoff = [0]
    boff = [0]
    saoff = [0]
    sboff = [0]
    for b in range(n_blocks):
        aoff.append(aoff[-1] + ia[b] + sa[b])
        boff.append(boff[-1] + ib[b] + sb[b])
        saoff.append(saoff[-1] + sa[b])
        sboff.append(sboff[-1] + sb[b])
    nch_a, nch_b = aoff[-1], boff[-1]
    nsa, nsb = saoff[-1], sboff[-1]
    na16 = ((nch_a + GQ - 1) // GQ * GQ) * P // 16
    nb16 = ((nch_b + GQ - 1) // GQ * GQ) * P // 16

    nc = bacc.Bacc("TRN2", target_bir_lowering=False, debug=False,
                   num_devices=n_cores, num_swdge_queues=NQ)

    xs = nc.dram_tensor("xs", [P, n_blocks * D], f32, kind="ExternalInput").ap()
    ga = nc.dram_tensor("ga", [P, na16], i16, kind="ExternalInput").ap()
    gb = nc.dram_tensor("gb", [P, nb16], i16, kind="ExternalInput").ap()
    sa_in = nc.dram_tensor("sa", [P, max(nsa, 1) * P], bf16,
                           kind="ExternalInput").ap()
    sb_in = nc.dram_tensor("sb", [P, max(nsb, 1) * P], bf16,
                           kind="ExternalInput").ap()
    idbf_in = nc.dram_tensor("idbf", [P, D], bf16, kind="ExternalInput").ap()
    dinvb = nc.dram_tensor("dinvb", [P, n_blocks], f32,
                           kind="ExternalInput").ap()
    ws = [nc.dram_tensor(f"w{l}", [D, D], f32, kind="ExternalInput").ap()
          for l in range(3)]
    brs = [nc.dram_tensor(f"br{l}", [P, D], f32, kind="ExternalInput").ap()
           for l in range(3)]
    grs = [nc.dram_tensor(f"gr{l}", [P, D], f32, kind="ExternalInput").ap()
           for l in range(2)]
    bers = [nc.dram_tensor(f"ber{l}", [P, D], f32, kind="ExternalInput").ap()
            for l in range(2)]
    out_t = nc.dram_tensor("out", [npc, D_OUT], f32, kind="ExternalOutput").ap()

    with tile.TileContext(nc) as tc, ExitStack() as ctx:
        singles = ctx.enter_context(tc.tile_pool(name="singles", bufs=1))
        hpool = ctx.enter_context(tc.tile_pool(name="h", bufs=2))
        ypool = ctx.enter_context(tc.tile_pool(name="y", bufs=2))
        apool = ctx.enter_context(tc.tile_pool(name="accsb", bufs=1))
        stage = ctx.enter_context(tc.tile_pool(name="stage", bufs=12))
        spool = ctx.enter_context(tc.tile_pool(name="spool", bufs=6))
        htp = ctx.enter_context(tc.tile_pool(name="htp", bufs=3))
        epi = ctx.enter_context(tc.tile_pool(name="epi", bufs=3))
        small = ctx.enter_context(tc.tile_pool(name="small", bufs=4))
        lnp = ctx.enter_context(tc.tile_pool(name="lnp", bufs=2))
        ps_t = ctx.enter_context(tc.tile_pool(name="ps_t", bufs=2, space="PSUM"))
        ps_z = ctx.enter_context(tc.tile_pool(name="ps_z", bufs=2, space="PSUM"))
        ps_a = ctx.enter_context(tc.tile_pool(name="ps_a", bufs=4, space="PSUM"))
        dram = ctx.enter_context(tc.tile_pool(name="dram", bufs=1, space="DRAM"))

        # constants
        ident = singles.tile([P, P], f32)
        make_identity(nc, ident[:])
        w_t, br_t, gr_t, ber_t = [], [], [], []
        for l in range(3):
            w_t.append(singles.tile([D, D], f32, tag=f"w{l}", name=f"w{l}_t"))
            nc.sync.dma_start(w_t[l][:], ws[l][:])
            br_t.append(singles.tile([P, D], f32, tag=f"br{l}",
                                     name=f"br{l}_t"))
            nc.sync.dma_start(br_t[l][:], brs[l][:])
        for l in range(2):
            gr_t.append(singles.tile([P, D], f32, tag=f"gr{l}",
                                     name=f"gr{l}_t"))
            nc.sync.dma_start(gr_t[l][:], grs[l][:])
            ber_t.append(singles.tile([P, D], f32, tag=f"ber{l}",
                                      name=f"ber{l}_t"))
            nc.sync.dma_start(ber_t[l][:], bers[l][:])
        dinv_t = singles.tile([P, n_blocks], f32)
        nc.sync.dma_start(dinv_t[:], dinvb[:])
        ga_t = singles.tile([P, na16], i16)
        nc.sync.dma_start(ga_t[:], ga[:])
        gb_t = singles.tile([P, nb16], i16)
        nc.sync.dma_start(gb_t[:], gb[:])
        eps_t = singles.tile([P, 1], f32)
        nc.vector.memset(eps_t[:], eps)
        idbf_t = singles.tile([P, D], bf16)
        nc.sync.dma_start(idbf_t[:], idbf_in[:])
        zr_t = singles.tile([1, D], bf16)
        nc.vector.memset(zr_t[:], 0.0)

        # xs is feature-major (lhsT-ready); load the a-half first so the
        # layer-0 AllGather can fire as early as possible
        xT = hpool.tile([P, n_blocks * D], f32, tag="h", name="xT")
        nc.sync.dma_start(xT[:, 0:blk_a * D], xs[:, 0:blk_a * D])
        nc.sync.dma_start(xT[:, blk_a * D:], xs[:, blk_a * D:])

        y_own_a = dram.tile([rows_a_pr, D], bf16)
        y_own_b = dram.tile([rows_b, D], bf16)
        # appended per-rank zero row (holes in identity rounds gather it)
        nc.sync.dma_start(y_own_a[split:split + 1, :], zr_t[:])
        tables_a = [dram.tile([rows_a_pr * n_cores, D], bf16,
                              addr_space="Shared", name=f"table_a{l}")
                    for l in range(n_layers)]
        tables_b = [dram.tile([rows_b * n_cores, D], bf16,
                              addr_space="Shared", name=f"table_b{l}")
                    for l in range(n_layers)]

        qn = [0]

        def gather(stage_tile, n_chunks, tab, gidx_t, col0):
            n_idx = n_chunks * P
            nc.gpsimd.dma_gather(
                out_ap=stage_tile[:, 0:n_chunks, :], in_ap=tab,
                idxs_ap=gidx_t[:, col0:col0 + n_idx // 16],
                num_idxs=n_idx, num_idxs_reg=n_idx, elem_size=D,
                single_packet=SINGLE_PACKET, queue_num=qn[0] % NQ)
            qn[0] += 1

        alu_add = Alu.add

        def z_blocks(layer, h_src, y_sb, b0, b1, transposed=False):
            """y[:, b0:b1 blocks] = dinv * (h_src @ W_layer), bf16.

            transposed=True: h_src is already feature-major (lhsT-ready).
            """
            for b in range(b0, b1):
                bs = slice(b * D, (b + 1) * D)
                if transposed:
                    hT_ap = h_src[:, bs]
                else:
                    tp = ps_t.tile([P, P], f32, tag="tp", name="tp")
                    nc.tensor.transpose(out=tp[:], in_=h_src[:, bs],
                                        identity=ident[:])
                    hT = htp.tile([P, P], f32, tag="hT", name="hT")
                    nc.scalar.copy(hT[:], tp[:])
                    hT_ap = hT[:]
                zp = ps_z.tile([P, D], f32, tag="zp", name="zp")
                nc.tensor.matmul(out=zp[:], lhsT=hT_ap, rhs=w_t[layer][:],
                                 start=True, stop=True)
                nc.scalar.activation(y_sb[:, bs], zp[:], Act.Copy,
                                     scale=dinv_t[:, b:b + 1])

        def fire_ag_a(y_sb, l):
            yv = y_own_a[0:split, :].rearrange("(p b) j -> p b j", b=blk_a)
            sv = y_sb[:, 0:blk_a * D].rearrange("p (b j) -> p b j", j=D)
            nc.sync.dma_start(yv, sv)
            nc.gpsimd.collective_compute(
                "AllGather", Alu.bypass,
                ins=[y_own_a[:].opt()], outs=[tables_a[l][:].opt()],
                replica_groups=[list(range(n_cores))])

        def fire_ag_b(y_sb, l):
            yv = y_own_b[:].rearrange("(p b) j -> p b j",
                                      b=n_blocks - blk_a)
            sv = y_sb[:, blk_a * D:].rearrange("p (b j) -> p b j", j=D)
            nc.sync.dma_start(yv, sv)
            nc.gpsimd.collective_compute(
                "AllGather", Alu.bypass,
                ins=[y_own_b[:].opt()], outs=[tables_b[l][:].opt()],
                replica_groups=[list(range(n_cores))])

        # layer-0 message table from the input features (feature-major)
        y_sb = ypool.tile([P, n_blocks * D], bf16, tag="y")
        z_blocks(0, xT, y_sb, 0, blk_a, transposed=True)
        fire_ag_a(y_sb, 0)
        z_blocks(0, xT, y_sb, blk_a, n_blocks, transposed=True)
        fire_ag_b(y_sb, 0)

        for layer in range(n_layers):
            W = D_OUT if layer == n_layers - 1 else D
            acc_sb = apool.tile([P, n_blocks * D], f32, tag="acc")

            g_tiles = {}
            s_tiles = {}

            def g_stage(flat_chunk, n_chunks_tot, tab, gidx_t, tag):
                g = flat_chunk // GQ
                if (tag, g) not in g_tiles:
                    n_in_g = min(GQ, n_chunks_tot - g * GQ)
                    t = stage.tile([P, GQ, D], bf16, tag="stg", name="stg")
                    gather(t, n_in_g, tab, gidx_t, g * GQ * P // 16)
                    g_tiles[(tag, g)] = t
                return g_tiles[(tag, g)][:, flat_chunk % GQ, 0:W]

            def s_for(s_flat, n_s_tot, s_dram, tag):
                gs = s_flat // SGQ
                if (tag, gs) not in s_tiles:
                    n_in_s = min(SGQ, n_s_tot - gs * SGQ)
                    st = spool.tile([P, SGQ * P], bf16, tag="sst",
                                    name="sst")
                    nc.sync.dma_start(
                        st[:, 0:n_in_s * P],
                        s_dram[:, gs * SGQ * P:(gs * SGQ + n_in_s) * P])
                    s_tiles[(tag, gs)] = st
                ci = s_flat % SGQ
                return s_tiles[(tag, gs)][:, ci * P:(ci + 1) * P]

            # ---- phase A: table_a chunks -> acc_sb (= A_a@y + y) ----
            pend_a = []

            def flush_a():
                bb, aa = pend_a.pop(0)
                bbs = slice(bb * D, bb * D + W)
                nc.vector.tensor_tensor(out=acc_sb[:, bbs], in0=aa[:],
                                        in1=y_sb[:, bbs], op=alu_add)

            for b in range(n_blocks):
                acc = ps_a.tile([P, W], f32, tag="pacc")
                tot = ia[b] + sa[b]
                for k in range(tot):
                    msg = g_stage(aoff[b] + k, nch_a, tables_a[layer][:],
                                  ga_t, "sta")
                    if k < ia[b]:
                        s_ap = idbf_t[:]
                    else:
                        s_ap = s_for(saoff[b] + k - ia[b], nsa, sa_in,
                                     "ssa")
                    nc.tensor.matmul(out=acc[:], lhsT=s_ap, rhs=msg,
                                     start=(k == 0), stop=(k == tot - 1))
                pend_a.append((b, acc))
                if len(pend_a) > 3:
                    flush_a()
            while pend_a:
                flush_a()

            # ---- phase B: table_b chunks + epilogue ----
            has_bias, has_g, has_be = flags[layer]
            if layer < 2:
                h_nxt = hpool.tile([P, n_blocks * D], f32, tag="h")
                # LayerNorm batch stats: mean/var per block, batched recip
                mvall = lnp.tile([P, 2 * n_blocks], f32, tag="mv")
                negmu = lnp.tile([P, 2 * n_blocks], f32, tag="nm")
                stdall = lnp.tile([P, n_blocks], f32, tag="sd")
                rall = lnp.tile([P, n_blocks], f32, tag="ra")
            pend_b = []

            def epilogue(b, acc):
                bs = slice(b * D, b * D + W)
                # v = acc_a + acc_b (+ y folded into acc_sb already)
                if layer < 2:
                    dst_ap = h_nxt[:, bs]
                else:
                    v_t = epi.tile([P, W], f32, tag="v", name="v_t")
                    dst_ap = v_t[:]
                nc.vector.tensor_tensor(out=dst_ap, in0=acc[:],
                                        in1=acc_sb[:, bs], op=alu_add)
                if has_bias:
                    # bias is per-feature: needs replicated tile + TT add
                    nc.scalar.activation(dst_ap, dst_ap, Act.Copy,
                                         scale=dinv_t[:, b:b + 1])
                    nc.vector.tensor_tensor(out=dst_ap, in0=dst_ap,
                                            in1=br_t[layer][:], op=alu_add)
                    if layer < 2:
                        nc.scalar.activation(dst_ap, dst_ap, Act.Relu)
                elif layer < 2:
                    nc.scalar.activation(dst_ap, dst_ap, Act.Relu,
                                         scale=dinv_t[:, b:b + 1])
                else:
                    nc.scalar.activation(dst_ap, dst_ap, Act.Copy,
                                         scale=dinv_t[:, b:b + 1])
                if layer < 2:
                    stats = small.tile([P, 6], f32, tag="st")
                    nc.vector.bn_stats(out=stats[:], in_=dst_ap)
                    nc.vector.bn_aggr(out=mvall[:, 2 * b:2 * b + 2],
                                      in_=stats[:])
                    nc.scalar.activation(stdall[:, b:b + 1],
                                         mvall[:, 2 * b + 1:2 * b + 2],
                                         Act.Sqrt, bias=eps_t[:])
                else:
                    nc.sync.dma_start(out_t[b * P:(b + 1) * P, :],
                                      dst_ap)

            def ln_apply(b0, b1):
                # batched: negmu = -mv (mean cols), rall = 1/stdall
                nc.vector.tensor_scalar(out=negmu[:, 2 * b0:2 * b1],
                                        in0=mvall[:, 2 * b0:2 * b1],
                                        scalar1=-1.0, scalar2=None,
                                        op0=Alu.mult)
                nc.vector.reciprocal(rall[:, b0:b1], stdall[:, b0:b1])
                for b in range(b0, b1):
                    bs = slice(b * D, (b + 1) * D)
                    nmr = small.tile([P, 1], f32, tag="nmr", name="nmr")
                    nc.vector.tensor_tensor(out=nmr[:],
                                            in0=negmu[:, 2 * b:2 * b + 1],
                                            in1=rall[:, b:b + 1],
                                            op=Alu.mult)
                    needs_post = has_g or has_be
                    if needs_post:
                        v_t = epi.tile([P, D], f32, tag="v", name="v_t")
                        apply_out = v_t[:]
                    else:
                        apply_out = h_nxt[:, bs]
                    nc.scalar.activation(apply_out, h_nxt[:, bs],
                                         Act.Identity, bias=nmr[:],
                                         scale=rall[:, b:b + 1])
                    if has_g and has_be:
                        nc.vector.tensor_tensor(out=apply_out, in0=apply_out,
                                                in1=gr_t[layer][:],
                                                op=Alu.mult)
                        nc.vector.tensor_tensor(out=h_nxt[:, bs],
                                                in0=apply_out,
                                                in1=ber_t[layer][:],
                                                op=alu_add)
                    elif has_g:
                        nc.vector.tensor_tensor(out=h_nxt[:, bs],
                                                in0=apply_out,
                                                in1=gr_t[layer][:],
                                                op=Alu.mult)
                    elif has_be:
                        nc.vector.tensor_tensor(out=h_nxt[:, bs],
                                                in0=apply_out,
                                                in1=ber_t[layer][:],
                                                op=alu_add)

            n_done = [0]
            y_nxt = None

            def pop_epilogue():
                nonlocal y_nxt
                epilogue(*pend_b.pop(0))
                n_done[0] += 1
                if n_done[0] == blk_a and layer < 2:
                    # first half of h_nxt finished: start next layer's
                    # message table + AllGather while phase B continues
                    ln_apply(0, blk_a)
                    y_nxt = ypool.tile([P, n_blocks * D], bf16, tag="y",
                                       name="y_nxt")
                    z_blocks(layer + 1, h_nxt, y_nxt, 0, blk_a)
                    fire_ag_a(y_nxt, layer + 1)

            for b in range(n_blocks):
                acc = ps_a.tile([P, W], f32, tag="pacc")
                tot = ib[b] + sb[b]
                for k in range(tot):
                    msg = g_stage(boff[b] + k, nch_b, tables_b[layer][:],
                                  gb_t, "stb")
                    if k < ib[b]:
                        s_ap = idbf_t[:]
                    else:
                        s_ap = s_for(sboff[b] + k - ib[b], nsb, sb_in,
                                     "ssb")
                    nc.tensor.matmul(out=acc[:], lhsT=s_ap, rhs=msg,
                                     start=(k == 0), stop=(k == tot - 1))
                pend_b.append((b, acc))
                if len(pend_b) > 3:
                    pop_epilogue()
            while pend_b:
                pop_epilogue()

            if layer < 2:
                ln_apply(blk_a, n_blocks)
                z_blocks(layer + 1, h_nxt, y_nxt, blk_a, n_blocks)
                fire_ag_b(y_nxt, layer + 1)
                h_cur = h_nxt
                y_sb = y_nxt

    nc.compile()
    return nc


# ----------------------------------------------------------------------------
# Full-size entry point
# ----------------------------------------------------------------------------

N_NODES = 50000
N_CORES = 8
N_BLOCKS = 49            # 49*128 = 6272 nodes per core, 50176 padded
BLK_A = 17               # blocks per rank in table_a (8*17*128 = 17408 rows;
                         # table_b 8*32*128 = 32768 rows, max int16 idx 32767)

_KERNEL_CACHE = {}


def make_input_maps(x, edge_index, W1, b1, W2, b2, W3, b3, g1, be1, g2, be2,
                    n_nodes, n_cores, n_blocks, blk_a):
    x = np.asarray(x, np.float32)
    pre = preprocess(np.asarray(edge_index), n_nodes, n_cores, n_blocks,
                     blk_a)
    xsh = shard_x(x, n_nodes, n_cores, n_blocks)
    w3p = np.zeros((D, D), np.float32)
    w3 = np.asarray(W3, np.float32)
    w3p[:, :w3.shape[1]] = w3
    b3p = np.zeros((D,), np.float32)
    b3a = np.asarray(b3, np.float32)
    b3p[:b3a.shape[0]] = b3a
    rep = lambda a: np.ascontiguousarray(
        np.tile(np.asarray(a, np.float32)[None, :], (P, 1)))

    bs = [np.asarray(b1, np.float32), np.asarray(b2, np.float32), b3p]
    gs = [np.asarray(g1, np.float32), np.asarray(g2, np.float32)]
    bes = [np.asarray(be1, np.float32), np.asarray(be2, np.float32)]
    flags = tuple(
        (bool(np.any(bs[l] != 0.0)),
         bool(l < 2 and np.any(gs[l] != 1.0)),
         bool(l < 2 and np.any(bes[l] != 0.0)))
        for l in range(3))
    pre["flags"] = flags

    # pad wrapped idx arrays up to the group-aligned width the kernel expects
    nch_a = sum(i + s for i, s in pre["CA"])
    nch_b = sum(i + s for i, s in pre["CB"])
    na16 = ((nch_a + GQ - 1) // GQ * GQ) * P // 16
    nb16 = ((nch_b + GQ - 1) // GQ * GQ) * P // 16

    def padw(a, w):
        if a.shape[1] < w:
            a = np.concatenate(
                [a, np.zeros((P, w - a.shape[1]), np.int16)], axis=1)
        return np.ascontiguousarray(a)

    import ml_dtypes
    idbf = np.ascontiguousarray(np.eye(P).astype(ml_dtypes.bfloat16))
    shared = {
        "idbf": idbf,
        "w0": np.asarray(W1, np.float32), "w1": np.asarray(W2, np.float32),
        "w2": w3p,
        "br0": rep(bs[0]), "br1": rep(bs[1]), "br2": rep(bs[2]),
        "gr0": rep(gs[0]), "gr1": rep(gs[1]),
        "ber0": rep(bes[0]), "ber1": rep(bes[1]),
    }
    in_maps = []
    for c in range(n_cores):
        pc = pre["cores"][c]
        in_maps.append({
            "xs": xsh[c], "ga": padw(pc["ga"], na16),
            "gb": padw(pc["gb"], nb16),
            "sa": pc["sa"], "sb": pc["sb"],
            "dinvb": pc["dinvb"], **shared,
        })
    return in_maps, pre


def kernel(x, edge_index, W1, b1, W2, b2, W3, b3, g1, be1, g2, be2):
    from concourse.bass_utils import run_bass_kernel_spmd

    in_maps, pre = make_input_maps(
        x, edge_index, W1, b1, W2, b2, W3, b3, g1, be1, g2, be2,
        N_NODES, N_CORES, N_BLOCKS, BLK_A)
    key = (N_CORES, N_BLOCKS, BLK_A, pre["CA"], pre["CB"], pre["flags"])
    if key not in _KERNEL_CACHE:
        _KERNEL_CACHE[key] = build_kernel(N_CORES, N_BLOCKS, BLK_A,
                                          pre["CA"], pre["CB"],
                                          pre["flags"])
    nc = _KERNEL_CACHE[key]

    res = run_bass_kernel_spmd(nc, in_maps, core_ids=list(range(N_CORES)))
    out = np.concatenate([res.results[c]["out"] for c in range(N_CORES)],
                         axis=0)
    return out[:N_NODES]


# revision 74
# speedup vs baseline: 1.3862x; 1.3862x over previous
"""3-layer GCN (gnn_message_passing) on 8 Trainium2 NeuronCores.

Sharding: nodes partitioned by range across 8 cores (dst-sharded).
Per layer, per core:
  1. z = h_shard @ W  (PE transpose per 128-node block + matmul),
     y = dinv * z  cast to bf16 (the "message table" values)
  2. Two AllGathers (first/second half of each rank's shard) -> two
     bf16 tables in DRAM; AG_a overlaps the second half of the z phase,
     AG_b overlaps phase-A gathers/matmuls.
  3. dma_gather the table rows for this core's in-edges, 2048 idxs per
     instruction spread over 4 SWDGE queues.
  4. segment-sum per 128-edge chunk via PE matmul with HOST-PRECOMPUTED
     one-hot S[edge, dst_local] tiles streamed from DRAM (S is graph
     structure, shared by all 3 layers; streaming it costs idle HBM BW
     instead of 1.3us/chunk of DVE time).
  5. epilogue per block: relu(dinv*(A@y + y)) in one ACT op, LayerNorm
     with batched reciprocal.
Self-loops are folded in via the "+ y" term (norm factorizes as
dinv[src]*dinv[dst]).
"""

import numpy as np
from contextlib import ExitStack

P = 128
D = 128          # feature width of layers (W3 zero-padded 64 -> 128)
D_OUT = 64
GQ = 8           # chunks per gather instruction (8*128 = 1024 idxs)
SGQ = 16         # chunks per one-hot S DMA load
SINGLE_PACKET = True   # 64 descs/engine, all 16 engines in parallel
NQ = 4           # SWDGE queues used round-robin


# ----------------------------------------------------------------------------
# Host-side graph preprocessing
# ----------------------------------------------------------------------------

def preprocess(edge_index, n_nodes, n_cores, n_blocks, blk_a, gq=GQ):
    """Build per-core gather/scatter index arrays + one-hot S tiles.

    Nodes are split into half-shards per rank: local rows [0, blk_a*128)
    go to table_a (AllGather #1), the rest to table_b.  Table rows are
    partition-major within a rank (node (b,p) -> row p*nblk+b) so the
    y_own DRAM write is one contiguous run per SBUF partition.  Chunk
    slots are sorted by table row for HBM locality in the gather.
    """
    npc = n_blocks * P
    split = blk_a * P                       # local row where half b starts
    rows_b = npc - split
    src = np.asarray(edge_index[0], dtype=np.int64)
    dst = np.asarray(edge_index[1], dtype=np.int64)

    deg = np.bincount(dst, minlength=n_nodes).astype(np.float32) + 1.0
    dinv = np.zeros(npc * n_cores, np.float32)
    dinv[:n_nodes] = 1.0 / np.sqrt(deg)

    r = src // npc
    off = src % npc
    bb = off // P
    pp = off % P
    in_a = off < split
    trow = np.where(in_a,
                    r * split + pp * blk_a + bb,
                    r * rows_b + pp * (n_blocks - blk_a) + (bb - blk_a))

    core_of = dst // npc
    per_core = []
    cnt_a = np.zeros((n_cores, n_blocks), np.int64)
    cnt_b = np.zeros((n_cores, n_blocks), np.int64)
    for c in range(n_cores):
        m = core_of == c
        s_t = trow[m]
        s_a = in_a[m]
        dl = dst[m] - c * npc
        blk = dl // P
        din = dl % P
        order = np.lexsort((s_t, ~s_a, blk))
        s_t, s_a, blk, din = s_t[order], s_a[order], blk[order], din[order]
        per_core.append((s_t, s_a, blk, din))
        cnt_a[c] = np.bincount(blk[s_a], minlength=n_blocks)
        cnt_b[c] = np.bincount(blk[~s_a], minlength=n_blocks)

    ca = ((cnt_a.max(axis=0) + P - 1) // P).astype(int)
    cb = ((cnt_b.max(axis=0) + P - 1) // P).astype(int)
    ca = np.maximum(ca, 1)
    cb = np.maximum(cb, 1)
    aoff = np.concatenate([[0], np.cumsum(ca)])
    boff = np.concatenate([[0], np.cumsum(cb)])
    nch_a, nch_b = int(aoff[-1]), int(boff[-1])
    out = {"CA": tuple(int(v) for v in ca), "CB": tuple(int(v) for v in cb),
           "dinv": dinv, "cores": []}
    eye = np.eye(P, dtype=np.float32)
    zrow = np.zeros((P,), np.float32)
    eye_pad = np.concatenate([eye, zrow[None, :]], axis=0)  # row P = zeros
    for c in range(n_cores):
        s_t, s_a, blk, din = per_core[c]
        gidx_a = np.zeros((nch_a, P), np.int64)   # dummy -> row 0
        gidx_b = np.zeros((nch_b, P), np.int64)
        dstid_a = np.full((nch_a, P), P, np.int64)   # P -> zero row of S
        dstid_b = np.full((nch_b, P), P, np.int64)
        for b in range(n_blocks):
            bm = blk == b
            ta, da = s_t[bm & s_a], din[bm & s_a]
            tb, db = s_t[bm & ~s_a], din[bm & ~s_a]
            gidx_a[aoff[b]:aoff[b + 1]].reshape(-1)[:len(ta)] = ta
            gidx_b[boff[b]:boff[b + 1]].reshape(-1)[:len(tb)] = tb
            dstid_a[aoff[b]:aoff[b + 1]].reshape(-1)[:len(da)] = da
            dstid_b[boff[b]:boff[b + 1]].reshape(-1)[:len(db)] = db

        def wrap(flat):
            # flat [chunks, 128]; groups of `gq` chunks per gather instr;
            # within an instr: idx i -> [i % 16, i // 16], replicated 8x.
            cols = []
            for g0 in range(0, flat.shape[0], gq):
                fg = flat[g0:g0 + gq].reshape(-1)
                w16 = fg.reshape(-1, 16).T
                cols.append(np.tile(w16, (8, 1)))
            return np.ascontiguousarray(
                np.concatenate(cols, axis=1).astype(np.int16))

        def onehot(dstid):
            # dstid [chunks, 128] -> S [128 edge, chunks*128 dst] bf16
            import ml_dtypes
            s = eye_pad[dstid]                     # [chunks, 128e, 128d]
            s = s.transpose(1, 0, 2).reshape(P, -1)
            return np.ascontiguousarray(s.astype(ml_dtypes.bfloat16))

        out["cores"].append({
            "ga": wrap(gidx_a),
            "gb": wrap(gidx_b),
            "sa": onehot(dstid_a),
            "sb": onehot(dstid_b),
            "dinvb": np.ascontiguousarray(
                dinv[c * npc:(c + 1) * npc].reshape(n_blocks, P).T),
        })
    return out


def shard_x(x, n_nodes, n_cores, n_blocks):
    """x [n,128] f32 -> per-core feature-major [128 feat, n_blocks*128].

    Per block b, columns b*128..(b+1)*128 hold x[block_nodes].T so the
    layer-0 matmul can use it as lhsT directly (no PE transpose).
    """
    npc = n_blocks * P
    xp = np.zeros((npc * n_cores, x.shape[1]), np.float32)
    xp[:n_nodes] = x
    shards = []
    for c in range(n_cores):
        xs = xp[c * npc:(c + 1) * npc].reshape(n_blocks, P, x.shape[1])
        shards.append(np.ascontiguousarray(
            xs.transpose(2, 0, 1).reshape(x.shape[1], n_blocks * P)))
    return shards


# ----------------------------------------------------------------------------
# Kernel builder
# ----------------------------------------------------------------------------

def build_kernel(n_cores, n_blocks, blk_a, ca, cb, flags, eps=1e-5,
                 n_layers=3):
    """flags: per-layer tuple of (has_bias, has_g, has_be)."""
    import concourse.bacc as bacc
    import concourse.mybir as mybir
    import concourse.tile as tile
    from concourse.masks import make_identity

    f32 = mybir.dt.float32
    bf16 = mybir.dt.bfloat16
    i16 = mybir.dt.int16
    Act = mybir.ActivationFunctionType
    Alu = mybir.AluOpType

    npc = n_blocks * P
    split = blk_a * P
    rows_b = npc - split
    ca = list(ca)
    cb = list(cb)
    aoff = [0]
    boff = [0]
    for b in range(n_blocks):
        aoff.append(aoff[-1] + ca[b])
        boff.append(boff[-1] + cb[b])
    nch_a, nch_b = aoff[-1], boff[-1]
    na16 = ((nch_a + GQ - 1) // GQ * GQ) * P // 16
    nb16 = ((nch_b + GQ - 1) // GQ * GQ) * P // 16

    nc = bacc.Bacc("TRN2", target_bir_lowering=False, debug=False,
                   num_devices=n_cores, num_swdge_queues=NQ)

    xs = nc.dram_tensor("xs", [P, n_blocks * D], f32, kind="ExternalInput").ap()
    ga = nc.dram_tensor("ga", [P, na16], i16, kind="ExternalInput").ap()
    gb = nc.dram_tensor("gb", [P, nb16], i16, kind="ExternalInput").ap()
    sa_in = nc.dram_tensor("sa", [P, nch_a * P], bf16,
                           kind="ExternalInput").ap()
    sb_in = nc.dram_tensor("sb", [P, nch_b * P], bf16,
                           kind="ExternalInput").ap()
    dinvb = nc.dram_tensor("dinvb", [P, n_blocks], f32,
                           kind="ExternalInput").ap()
    ws = [nc.dram_tensor(f"w{l}", [D, D], f32, kind="ExternalInput").ap()
          for l in range(3)]
    brs = [nc.dram_tensor(f"br{l}", [P, D], f32, kind="ExternalInput").ap()
           for l in range(3)]
    grs = [nc.dram_tensor(f"gr{l}", [P, D], f32, kind="ExternalInput").ap()
           for l in range(2)]
    bers = [nc.dram_tensor(f"ber{l}", [P, D], f32, kind="ExternalInput").ap()
            for l in range(2)]
    out_t = nc.dram_tensor("out", [npc, D_OUT], f32, kind="ExternalOutput").ap()

    with tile.TileContext(nc) as tc, ExitStack() as ctx:
        singles = ctx.enter_context(tc.tile_pool(name="singles", bufs=1))
        hpool = ctx.enter_context(tc.tile_pool(name="h", bufs=2))
        ypool = ctx.enter_context(tc.tile_pool(name="y", bufs=2))
        apool = ctx.enter_context(tc.tile_pool(name="accsb", bufs=1))
        stage = ctx.enter_context(tc.tile_pool(name="stage", bufs=12))
        spool = ctx.enter_context(tc.tile_pool(name="spool", bufs=6))
        htp = ctx.enter_context(tc.tile_pool(name="htp", bufs=3))
        epi = ctx.enter_context(tc.tile_pool(name="epi", bufs=3))
        small = ctx.enter_context(tc.tile_pool(name="small", bufs=4))
        lnp = ctx.enter_context(tc.tile_pool(name="lnp", bufs=2))
        ps_t = ctx.enter_context(tc.tile_pool(name="ps_t", bufs=2, space="PSUM"))
        ps_z = ctx.enter_context(tc.tile_pool(name="ps_z", bufs=2, space="PSUM"))
        ps_a = ctx.enter_context(tc.tile_pool(name="ps_a", bufs=4, space="PSUM"))
        dram = ctx.enter_context(tc.tile_pool(name="dram", bufs=1, space="DRAM"))

        # constants
        ident = singles.tile([P, P], f32)
        make_identity(nc, ident[:])
        w_t, br_t, gr_t, ber_t = [], [], [], []
        for l in range(3):
            w_t.append(singles.tile([D, D], f32, tag=f"w{l}", name=f"w{l}_t"))
            nc.sync.dma_start(w_t[l][:], ws[l][:])
            br_t.append(singles.tile([P, D], f32, tag=f"br{l}",
                                     name=f"br{l}_t"))
            nc.sync.dma_start(br_t[l][:], brs[l][:])
        for l in range(2):
            gr_t.append(singles.tile([P, D], f32, tag=f"gr{l}",
                                     name=f"gr{l}_t"))
            nc.sync.dma_start(gr_t[l][:], grs[l][:])
            ber_t.append(singles.tile([P, D], f32, tag=f"ber{l}",
                                      name=f"ber{l}_t"))
            nc.sync.dma_start(ber_t[l][:], bers[l][:])
        dinv_t = singles.tile([P, n_blocks], f32)
        nc.sync.dma_start(dinv_t[:], dinvb[:])
        ga_t = singles.tile([P, na16], i16)
        nc.sync.dma_start(ga_t[:], ga[:])
        gb_t = singles.tile([P, nb16], i16)
        nc.sync.dma_start(gb_t[:], gb[:])
        eps_t = singles.tile([P, 1], f32)
        nc.vector.memset(eps_t[:], eps)

        # xs is feature-major (lhsT-ready); load the a-half first so the
        # layer-0 AllGather can fire as early as possible
        xT = hpool.tile([P, n_blocks * D], f32, tag="h", name="xT")
        nc.sync.dma_start(xT[:, 0:blk_a * D], xs[:, 0:blk_a * D])
        nc.sync.dma_start(xT[:, blk_a * D:], xs[:, blk_a * D:])

        y_own_a = dram.tile([split, D], bf16)
        y_own_b = dram.tile([rows_b, D], bf16)
        tables_a = [dram.tile([split * n_cores, D], bf16,
                              addr_space="Shared", name=f"table_a{l}")
                    for l in range(n_layers)]
        tables_b = [dram.tile([rows_b * n_cores, D], bf16,
                              addr_space="Shared", name=f"table_b{l}")
                    for l in range(n_layers)]

        qn = [0]

        def gather(stage_tile, n_chunks, tab, gidx_t, col0):
            n_idx = n_chunks * P
            nc.gpsimd.dma_gather(
                out_ap=stage_tile[:, 0:n_chunks, :], in_ap=tab,
                idxs_ap=gidx_t[:, col0:col0 + n_idx // 16],
                num_idxs=n_idx, num_idxs_reg=n_idx, elem_size=D,
                single_packet=SINGLE_PACKET, queue_num=qn[0] % NQ)
            qn[0] += 1

        alu_add = Alu.add

        def z_blocks(layer, h_src, y_sb, b0, b1, transposed=False):
            """y[:, b0:b1 blocks] = dinv * (h_src @ W_layer), bf16.

            transposed=True: h_src is already feature-major (lhsT-ready).
            """
            for b in range(b0, b1):
                bs = slice(b * D, (b + 1) * D)
                if transposed:
                    hT_ap = h_src[:, bs]
                else:
                    tp = ps_t.tile([P, P], f32, tag="tp", name="tp")
                    nc.tensor.transpose(out=tp[:], in_=h_src[:, bs],
                                        identity=ident[:])
                    hT = htp.tile([P, P], f32, tag="hT", name="hT")
                    nc.scalar.copy(hT[:], tp[:])
                    hT_ap = hT[:]
                zp = ps_z.tile([P, D], f32, tag="zp", name="zp")
                nc.tensor.matmul(out=zp[:], lhsT=hT_ap, rhs=w_t[layer][:],
                                 start=True, stop=True)
                nc.scalar.activation(y_sb[:, bs], zp[:], Act.Copy,
                                     scale=dinv_t[:, b:b + 1])

        def fire_ag_a(y_sb, l):
            yv = y_own_a[:].rearrange("(p b) j -> p b j", b=blk_a)
            sv = y_sb[:, 0:blk_a * D].rearrange("p (b j) -> p b j", j=D)
            nc.sync.dma_start(yv, sv)
            nc.gpsimd.collective_compute(
                "AllGather", Alu.bypass,
                ins=[y_own_a[:].opt()], outs=[tables_a[l][:].opt()],
                replica_groups=[list(range(n_cores))])

        def fire_ag_b(y_sb, l):
            yv = y_own_b[:].rearrange("(p b) j -> p b j",
                                      b=n_blocks - blk_a)
            sv = y_sb[:, blk_a * D:].rearrange("p (b j) -> p b j", j=D)
            nc.sync.dma_start(yv, sv)
            nc.gpsimd.collective_compute(
                "AllGather", Alu.bypass,
                ins=[y_own_b[:].opt()], outs=[tables_b[l][:].opt()],
                replica_groups=[list(range(n_cores))])

        # layer-0 message table from the input features (feature-major)
        y_sb = ypool.tile([P, n_blocks * D], bf16, tag="y")
        z_blocks(0, xT, y_sb, 0, blk_a, transposed=True)
        fire_ag_a(y_sb, 0)
        z_blocks(0, xT, y_sb, blk_a, n_blocks, transposed=True)
        fire_ag_b(y_sb, 0)

        for layer in range(n_layers):
            W = D_OUT if layer == n_layers - 1 else D
            acc_sb = apool.tile([P, n_blocks * D], f32, tag="acc")

            g_tiles = {}
            s_tiles = {}

            def stage_for(flat_chunk, n_chunks_tot, tab, gidx_t, s_dram,
                          tag):
                g = flat_chunk // GQ
                if (tag, g) not in g_tiles:
                    n_in_g = min(GQ, n_chunks_tot - g * GQ)
                    t = stage.tile([P, GQ, D], bf16, tag="stg", name="stg")
                    gather(t, n_in_g, tab, gidx_t, g * GQ * P // 16)
                    g_tiles[(tag, g)] = t
                gs = flat_chunk // SGQ
                if (tag, gs) not in s_tiles:
                    n_in_s = min(SGQ, n_chunks_tot - gs * SGQ)
                    st = spool.tile([P, SGQ * P], bf16, tag="sst",
                                    name="sst")
                    nc.sync.dma_start(
                        st[:, 0:n_in_s * P],
                        s_dram[:, gs * SGQ * P:(gs * SGQ + n_in_s) * P])
                    s_tiles[(tag, gs)] = st
                msg = g_tiles[(tag, g)][:, flat_chunk % GQ, 0:W]
                ci = flat_chunk % SGQ
                s_ap = s_tiles[(tag, gs)][:, ci * P:(ci + 1) * P]
                return msg, s_ap

            # ---- phase A: table_a chunks -> acc_sb (= A_a@y + y) ----
            pend_a = []

            def flush_a():
                bb, aa = pend_a.pop(0)
                bbs = slice(bb * D, bb * D + W)
                nc.vector.tensor_tensor(out=acc_sb[:, bbs], in0=aa[:],
                                        in1=y_sb[:, bbs], op=alu_add)

            for b in range(n_blocks):
                acc = ps_a.tile([P, W], f32, tag="pacc")
                for k in range(ca[b]):
                    msg, s_ap = stage_for(aoff[b] + k, nch_a,
                                          tables_a[layer][:],
                                          ga_t, sa_in, "sta")
                    nc.tensor.matmul(out=acc[:], lhsT=s_ap, rhs=msg,
                                     start=(k == 0), stop=(k == ca[b] - 1))
                pend_a.append((b, acc))
                if len(pend_a) > 3:
                    flush_a()
            while pend_a:
                flush_a()

            # ---- phase B: table_b chunks + epilogue ----
            has_bias, has_g, has_be = flags[layer]
            if layer < 2:
                h_nxt = hpool.tile([P, n_blocks * D], f32, tag="h")
                # LayerNorm batch stats: mean/var per block, batched recip
                mvall = lnp.tile([P, 2 * n_blocks], f32, tag="mv")
                negmu = lnp.tile([P, 2 * n_blocks], f32, tag="nm")
                stdall = lnp.tile([P, n_blocks], f32, tag="sd")
                rall = lnp.tile([P, n_blocks], f32, tag="ra")
            pend_b = []

            def epilogue(b, acc):
                bs = slice(b * D, b * D + W)
                # v = acc_a + acc_b (+ y folded into acc_sb already)
                if layer < 2:
                    dst_ap = h_nxt[:, bs]
                else:
                    v_t = epi.tile([P, W], f32, tag="v", name="v_t")
                    dst_ap = v_t[:]
                nc.vector.tensor_tensor(out=dst_ap, in0=acc[:],
                                        in1=acc_sb[:, bs], op=alu_add)
                if has_bias:
                    # bias is per-feature: needs replicated tile + TT add
                    nc.scalar.activation(dst_ap, dst_ap, Act.Copy,
                                         scale=dinv_t[:, b:b + 1])
                    nc.vector.tensor_tensor(out=dst_ap, in0=dst_ap,
                                            in1=br_t[layer][:], op=alu_add)
                    if layer < 2:
                        nc.scalar.activation(dst_ap, dst_ap, Act.Relu)
                elif layer < 2:
                    nc.scalar.activation(dst_ap, dst_ap, Act.Relu,
                                         scale=dinv_t[:, b:b + 1])
                else:
                    nc.scalar.activation(dst_ap, dst_ap, Act.Copy,
                                         scale=dinv_t[:, b:b + 1])
                if layer < 2:
                    stats = small.tile([P, 6], f32, tag="st")
                    nc.vector.bn_stats(out=stats[:], in_=dst_ap)
                    nc.vector.bn_aggr(out=mvall[:, 2 * b:2 * b + 2],
                                      in_=stats[:])
                    nc.scalar.activation(stdall[:, b:b + 1],
                                         mvall[:, 2 * b + 1:2 * b + 2],
                                         Act.Sqrt, bias=eps_t[:])
                else:
                    nc.sync.dma_start(out_t[b * P:(b + 1) * P, :],
                                      dst_ap)

            def ln_apply(b0, b1):
                # batched: negmu = -mv (mean cols), rall = 1/stdall
                nc.vector.tensor_scalar(out=negmu[:, 2 * b0:2 * b1],
                                        in0=mvall[:, 2 * b0:2 * b1],
                                        scalar1=-1.0, scalar2=None,
                                        op0=Alu.mult)
                nc.vector.reciprocal(rall[:, b0:b1], stdall[:, b0:b1])
                for b in range(b0, b1):
                    bs = slice(b * D, (b + 1) * D)
                    nmr = small.tile([P, 1], f32, tag="nmr", name="nmr")
                    nc.vector.tensor_tensor(out=nmr[:],
                                            in0=negmu[:, 2 * b:2 * b + 1],
                                            in1=rall[:, b:b + 1],
                                            op=Alu.mult)
                    needs_post = has_g or has_be
                    if needs_post:
                        v_t = epi.tile([P, D], f32, tag="v", name="v_t")
                        apply_out = v_t[:]
                    else:
                        apply_out = h_nxt[:, bs]
                    nc.scalar.activation(apply_out, h_nxt[:, bs],
                                         Act.Identity, bias=nmr[:],
                                         scale=rall[:, b:b + 1])
                    if has_g and has_be:
                        nc.vector.tensor_tensor(out=apply_out, in0=apply_out,
                                                in1=gr_t[layer][:],
                                                op=Alu.mult)
                        nc.vector.tensor_tensor(out=h_nxt[:, bs],
                                                in0=apply_out,
                                                in1=ber_t[layer][:],
                                                op=alu_add)
                    elif has_g:
                        nc.vector.tensor_tensor(out=h_nxt[:, bs],
                                                in0=apply_out,
                                                in1=gr_t[layer][:],
                                                op=Alu.mult)
                    elif has_be:
                        nc.vector.tensor_tensor(out=h_nxt[:, bs],
                                                in0=apply_out,
                                                in1=ber_t[layer][:],
                                                op=alu_add)

            n_done = [0]
            y_nxt = None

            def pop_epilogue():
                nonlocal y_nxt
                epilogue(*pend_b.pop(0))
                n_done[0] += 1
                if n_done[0] == blk_a and layer < 2:
                    # first half of h_nxt finished: start next layer's
                    # message table + AllGather while phase B continues
                    ln_apply(0, blk_a)
                    y_nxt = ypool.tile([P, n_blocks * D], bf16, tag="y",
                                       name="y_nxt")
                    z_blocks(layer + 1, h_nxt, y_nxt, 0, blk_a)
                    fire_ag_a(y_nxt, layer + 1)

            for b in range(n_blocks):
                acc = ps_a.tile([P, W], f32, tag="pacc")
                for k in range(cb[b]):
                    msg, s_ap = stage_for(boff[b] + k, nch_b,
                                          tables_b[layer][:],
                                          gb_t, sb_in, "stb")
                    nc.tensor.matmul(out=acc[:], lhsT=s_ap, rhs=msg,
                                     start=(k == 0), stop=(k == cb[b] - 1))
                pend_b.append((b, acc))
                if len(pend_b) > 3:
                    pop_epilogue()
            while pend_b:
                pop_epilogue()

            if layer < 2:
                ln_apply(blk_a, n_blocks)
                z_blocks(layer + 1, h_nxt, y_nxt, blk_a, n_blocks)
                fire_ag_b(y_nxt, layer + 1)
                h_cur = h_nxt
                y_sb = y_nxt

    nc.compile()
    return nc


# ----------------------------------------------------------------------------
# Full-size entry point
# ----------------------------------------------------------------------------

N_NODES = 50000
N_CORES = 8
N_BLOCKS = 49            # 49*128 = 6272 nodes per core, 50176 padded
BLK_A = 17               # blocks per rank in table_a (8*17*128 = 17408 rows;
                         # table_b 8*32*128 = 32768 rows, max int16 idx 32767)

_KERNEL_CACHE = {}


def make_input_maps(x, edge_index, W1, b1, W2, b2, W3, b3, g1, be1, g2, be2,
                    n_nodes, n_cores, n_blocks, blk_a):
    x = np.asarray(x, np.float32)
    pre = preprocess(np.asarray(edge_index), n_nodes, n_cores, n_blocks,
                     blk_a)
    xsh = shard_x(x, n_nodes, n_cores, n_blocks)
    w3p = np.zeros((D, D), np.float32)
    w3 = np.asarray(W3, np.float32)
    w3p[:, :w3.shape[1]] = w3
    b3p = np.zeros((D,), np.float32)
    b3a = np.asarray(b3, np.float32)
    b3p[:b3a.shape[0]] = b3a
    rep = lambda a: np.ascontiguousarray(
        np.tile(np.asarray(a, np.float32)[None, :], (P, 1)))

    bs = [np.asarray(b1, np.float32), np.asarray(b2, np.float32), b3p]
    gs = [np.asarray(g1, np.float32), np.asarray(g2, np.float32)]
    bes = [np.asarray(be1, np.float32), np.asarray(be2, np.float32)]
    flags = tuple(
        (bool(np.any(bs[l] != 0.0)),
         bool(l < 2 and np.any(gs[l] != 1.0)),
         bool(l < 2 and np.any(bes[l] != 0.0)))
        for l in range(3))
    pre["flags"] = flags

    # pad wrapped idx arrays up to the group-aligned width the kernel expects
    nch_a = sum(pre["CA"])
    nch_b = sum(pre["CB"])
    na16 = ((nch_a + GQ - 1) // GQ * GQ) * P // 16
    nb16 = ((nch_b + GQ - 1) // GQ * GQ) * P // 16

    def padw(a, w):
        if a.shape[1] < w:
            a = np.concatenate(
                [a, np.zeros((P, w - a.shape[1]), np.int16)], axis=1)
        return np.ascontiguousarray(a)

    shared = {
        "w0": np.asarray(W1, np.float32), "w1": np.asarray(W2, np.float32),
        "w2": w3p,
        "br0": rep(bs[0]), "br1": rep(bs[1]), "br2": rep(bs[2]),
        "gr0": rep(gs[0]), "gr1": rep(gs[1]),
        "ber0": rep(bes[0]), "ber1": rep(bes[1]),
    }
    in_maps = []
    for c in range(n_cores):
        pc = pre["cores"][c]
        in_maps.append({
            "xs": xsh[c], "ga": padw(pc["ga"], na16),
            "gb": padw(pc["gb"], nb16),
            "sa": pc["sa"], "sb": pc["sb"],
            "dinvb": pc["dinvb"], **shared,
        })
    return in_maps, pre


def kernel(x, edge_index, W1, b1, W2, b2, W3, b3, g1, be1, g2, be2):
    from concourse.bass_utils import run_bass_kernel_spmd

    in_maps, pre = make_input_maps(
        x, edge_index, W1, b1, W2, b2, W3, b3, g1, be1, g2, be2,
        N_NODES, N_CORES, N_BLOCKS, BLK_A)
    key = (N_CORES, N_BLOCKS, BLK_A, pre["CA"], pre["CB"], pre["flags"])
    if key not in _KERNEL_CACHE:
        _KERNEL_CACHE[key] = build_kernel(N_CORES, N_BLOCKS, BLK_A,
                                          pre["CA"], pre["CB"],
                                          pre["flags"])
    nc = _KERNEL_CACHE[key]

    res = run_bass_kernel_spmd(nc, in_maps, core_ids=list(range(N_CORES)))
    out = np.concatenate([res.results[c]["out"] for c in range(N_CORES)],
                         axis=0)
    return out[:N_NODES]


# revision 76
# speedup vs baseline: 1.4451x; 1.0425x over previous
"""3-layer GCN (gnn_message_passing) on 8 Trainium2 NeuronCores.

Sharding: nodes partitioned by range across 8 cores (dst-sharded).
Per layer, per core:
  1. z = h_shard @ W  (PE transpose per 128-node block + matmul),
     y = dinv * z  cast to bf16 (the "message table" values)
  2. Two AllGathers (first/second half of each rank's shard) -> two
     bf16 tables in DRAM; AG_a overlaps the second half of the z phase,
     AG_b overlaps phase-A gathers/matmuls.
  3. dma_gather the table rows for this core's in-edges, 2048 idxs per
     instruction spread over 4 SWDGE queues.
  4. segment-sum per 128-edge chunk via PE matmul with HOST-PRECOMPUTED
     one-hot S[edge, dst_local] tiles streamed from DRAM (S is graph
     structure, shared by all 3 layers; streaming it costs idle HBM BW
     instead of 1.3us/chunk of DVE time).
  5. epilogue per block: relu(dinv*(A@y + y)) in one ACT op, LayerNorm
     with batched reciprocal.
Self-loops are folded in via the "+ y" term (norm factorizes as
dinv[src]*dinv[dst]).
"""

import numpy as np
from contextlib import ExitStack

P = 128
D = 128          # feature width of layers (W3 zero-padded 64 -> 128)
D_OUT = 64
GQ = 8           # chunks per gather instruction (8*128 = 1024 idxs)
SGQ = 16         # chunks per one-hot S DMA load
SINGLE_PACKET = True   # 64 descs/engine, all 16 engines in parallel
NQ = 4           # SWDGE queues used round-robin


# ----------------------------------------------------------------------------
# Host-side graph preprocessing
# ----------------------------------------------------------------------------

def preprocess(edge_index, n_nodes, n_cores, n_blocks, blk_a, gq=GQ):
    """Build per-core gather/scatter index arrays + one-hot S tiles.

    Nodes are split into half-shards per rank: local rows [0, blk_a*128)
    go to table_a (AllGather #1), the rest to table_b.  Table rows are
    partition-major within a rank (node (b,p) -> row p*nblk+b) so the
    y_own DRAM write is one contiguous run per SBUF partition.  Chunk
    slots are sorted by table row for HBM locality in the gather.
    """
    npc = n_blocks * P
    split = blk_a * P                       # local row where half b starts
    rows_b = npc - split
    src = np.asarray(edge_index[0], dtype=np.int64)
    dst = np.asarray(edge_index[1], dtype=np.int64)

    deg = np.bincount(dst, minlength=n_nodes).astype(np.float32) + 1.0
    dinv = np.zeros(npc * n_cores, np.float32)
    dinv[:n_nodes] = 1.0 / np.sqrt(deg)

    r = src // npc
    off = src % npc
    bb = off // P
    pp = off % P
    in_a = off < split
    trow = np.where(in_a,
                    r * split + pp * blk_a + bb,
                    r * rows_b + pp * (n_blocks - blk_a) + (bb - blk_a))

    core_of = dst // npc
    per_core = []
    cnt_a = np.zeros((n_cores, n_blocks), np.int64)
    cnt_b = np.zeros((n_cores, n_blocks), np.int64)
    for c in range(n_cores):
        m = core_of == c
        s_t = trow[m]
        s_a = in_a[m]
        dl = dst[m] - c * npc
        blk = dl // P
        din = dl % P
        order = np.lexsort((s_t, ~s_a, blk))
        s_t, s_a, blk, din = s_t[order], s_a[order], blk[order], din[order]
        per_core.append((s_t, s_a, blk, din))
        cnt_a[c] = np.bincount(blk[s_a], minlength=n_blocks)
        cnt_b[c] = np.bincount(blk[~s_a], minlength=n_blocks)

    ca = ((cnt_a.max(axis=0) + P - 1) // P).astype(int)
    cb = ((cnt_b.max(axis=0) + P - 1) // P).astype(int)
    ca = np.maximum(ca, 1)
    cb = np.maximum(cb, 1)
    aoff = np.concatenate([[0], np.cumsum(ca)])
    boff = np.concatenate([[0], np.cumsum(cb)])
    nch_a, nch_b = int(aoff[-1]), int(boff[-1])
    out = {"CA": tuple(int(v) for v in ca), "CB": tuple(int(v) for v in cb),
           "dinv": dinv, "cores": []}
    eye = np.eye(P, dtype=np.float32)
    zrow = np.zeros((P,), np.float32)
    eye_pad = np.concatenate([eye, zrow[None, :]], axis=0)  # row P = zeros
    # slot permutation: put sorted-run ranks (j%16)*8 + j//16 at slot j so
    # SDMA engine k (slots j == k mod 16) sees 8 CONSECUTIVE sorted table
    # rows per chunk (HBM page locality) instead of a stride-16 subsample
    sigma = (np.arange(P) % 16) * 8 + np.arange(P) // 16
    for c in range(n_cores):
        s_t, s_a, blk, din = per_core[c]
        gidx_a = np.zeros((nch_a, P), np.int64)   # dummy -> row 0
        gidx_b = np.zeros((nch_b, P), np.int64)
        dstid_a = np.full((nch_a, P), P, np.int64)   # P -> zero row of S
        dstid_b = np.full((nch_b, P), P, np.int64)
        for b in range(n_blocks):
            bm = blk == b
            ta, da = s_t[bm & s_a], din[bm & s_a]
            tb, db = s_t[bm & ~s_a], din[bm & ~s_a]
            gidx_a[aoff[b]:aoff[b + 1]].reshape(-1)[:len(ta)] = ta
            gidx_b[boff[b]:boff[b + 1]].reshape(-1)[:len(tb)] = tb
            dstid_a[aoff[b]:aoff[b + 1]].reshape(-1)[:len(da)] = da
            dstid_b[boff[b]:boff[b + 1]].reshape(-1)[:len(db)] = db
        gidx_a = gidx_a[:, sigma]
        gidx_b = gidx_b[:, sigma]
        dstid_a = dstid_a[:, sigma]
        dstid_b = dstid_b[:, sigma]

        def wrap(flat):
            # flat [chunks, 128]; groups of `gq` chunks per gather instr;
            # within an instr: idx i -> [i % 16, i // 16], replicated 8x.
            cols = []
            for g0 in range(0, flat.shape[0], gq):
                fg = flat[g0:g0 + gq].reshape(-1)
                w16 = fg.reshape(-1, 16).T
                cols.append(np.tile(w16, (8, 1)))
            return np.ascontiguousarray(
                np.concatenate(cols, axis=1).astype(np.int16))

        def onehot(dstid):
            # dstid [chunks, 128] -> S [128 edge, chunks*128 dst] bf16
            import ml_dtypes
            s = eye_pad[dstid]                     # [chunks, 128e, 128d]
            s = s.transpose(1, 0, 2).reshape(P, -1)
            return np.ascontiguousarray(s.astype(ml_dtypes.bfloat16))

        out["cores"].append({
            "ga": wrap(gidx_a),
            "gb": wrap(gidx_b),
            "sa": onehot(dstid_a),
            "sb": onehot(dstid_b),
            "dinvb": np.ascontiguousarray(
                dinv[c * npc:(c + 1) * npc].reshape(n_blocks, P).T),
        })
    return out


def shard_x(x, n_nodes, n_cores, n_blocks):
    """x [n,128] f32 -> per-core feature-major [128 feat, n_blocks*128].

    Per block b, columns b*128..(b+1)*128 hold x[block_nodes].T so the
    layer-0 matmul can use it as lhsT directly (no PE transpose).
    """
    npc = n_blocks * P
    xp = np.zeros((npc * n_cores, x.shape[1]), np.float32)
    xp[:n_nodes] = x
    shards = []
    for c in range(n_cores):
        xs = xp[c * npc:(c + 1) * npc].reshape(n_blocks, P, x.shape[1])
        shards.append(np.ascontiguousarray(
            xs.transpose(2, 0, 1).reshape(x.shape[1], n_blocks * P)))
    return shards


# ----------------------------------------------------------------------------
# Kernel builder
# ----------------------------------------------------------------------------

def build_kernel(n_cores, n_blocks, blk_a, ca, cb, flags, eps=1e-5,
                 n_layers=3):
    """flags: per-layer tuple of (has_bias, has_g, has_be)."""
    import concourse.bacc as bacc
    import concourse.mybir as mybir
    import concourse.tile as tile
    from concourse.masks import make_identity

    f32 = mybir.dt.float32
    bf16 = mybir.dt.bfloat16
    i16 = mybir.dt.int16
    Act = mybir.ActivationFunctionType
    Alu = mybir.AluOpType

    npc = n_blocks * P
    split = blk_a * P
    rows_b = npc - split
    ca = list(ca)
    cb = list(cb)
    aoff = [0]
    boff = [0]
    for b in range(n_blocks):
        aoff.append(aoff[-1] + ca[b])
        boff.append(boff[-1] + cb[b])
    nch_a, nch_b = aoff[-1], boff[-1]
    na16 = ((nch_a + GQ - 1) // GQ * GQ) * P // 16
    nb16 = ((nch_b + GQ - 1) // GQ * GQ) * P // 16

    nc = bacc.Bacc("TRN2", target_bir_lowering=False, debug=False,
                   num_devices=n_cores, num_swdge_queues=NQ)

    xs = nc.dram_tensor("xs", [P, n_blocks * D], f32, kind="ExternalInput").ap()
    ga = nc.dram_tensor("ga", [P, na16], i16, kind="ExternalInput").ap()
    gb = nc.dram_tensor("gb", [P, nb16], i16, kind="ExternalInput").ap()
    sa_in = nc.dram_tensor("sa", [P, nch_a * P], bf16,
                           kind="ExternalInput").ap()
    sb_in = nc.dram_tensor("sb", [P, nch_b * P], bf16,
                           kind="ExternalInput").ap()
    dinvb = nc.dram_tensor("dinvb", [P, n_blocks], f32,
                           kind="ExternalInput").ap()
    ws = [nc.dram_tensor(f"w{l}", [D, D], f32, kind="ExternalInput").ap()
          for l in range(3)]
    brs = [nc.dram_tensor(f"br{l}", [P, D], f32, kind="ExternalInput").ap()
           for l in range(3)]
    grs = [nc.dram_tensor(f"gr{l}", [P, D], f32, kind="ExternalInput").ap()
           for l in range(2)]
    bers = [nc.dram_tensor(f"ber{l}", [P, D], f32, kind="ExternalInput").ap()
            for l in range(2)]
    out_t = nc.dram_tensor("out", [npc, D_OUT], f32, kind="ExternalOutput").ap()

    with tile.TileContext(nc) as tc, ExitStack() as ctx:
        singles = ctx.enter_context(tc.tile_pool(name="singles", bufs=1))
        hpool = ctx.enter_context(tc.tile_pool(name="h", bufs=2))
        ypool = ctx.enter_context(tc.tile_pool(name="y", bufs=2))
        apool = ctx.enter_context(tc.tile_pool(name="accsb", bufs=1))
        stage = ctx.enter_context(tc.tile_pool(name="stage", bufs=12))
        spool = ctx.enter_context(tc.tile_pool(name="spool", bufs=6))
        htp = ctx.enter_context(tc.tile_pool(name="htp", bufs=3))
        epi = ctx.enter_context(tc.tile_pool(name="epi", bufs=3))
        small = ctx.enter_context(tc.tile_pool(name="small", bufs=4))
        lnp = ctx.enter_context(tc.tile_pool(name="lnp", bufs=2))
        ps_t = ctx.enter_context(tc.tile_pool(name="ps_t", bufs=2, space="PSUM"))
        ps_z = ctx.enter_context(tc.tile_pool(name="ps_z", bufs=2, space="PSUM"))
        ps_a = ctx.enter_context(tc.tile_pool(name="ps_a", bufs=4, space="PSUM"))
        dram = ctx.enter_context(tc.tile_pool(name="dram", bufs=1, space="DRAM"))

        # prologue-critical loads first: xs a-half, w0, dinv gate the
        # layer-0 z phase and first AllGather
        xT = hpool.tile([P, n_blocks * D], f32, tag="h", name="xT")
        nc.sync.dma_start(xT[:, 0:blk_a * D], xs[:, 0:blk_a * D])
        w_t = []
        for l in range(3):
            w_t.append(singles.tile([D, D], f32, tag=f"w{l}", name=f"w{l}_t"))
        nc.sync.dma_start(w_t[0][:], ws[0][:])
        dinv_t = singles.tile([P, n_blocks], f32)
        nc.sync.dma_start(dinv_t[:], dinvb[:])
        nc.sync.dma_start(xT[:, blk_a * D:], xs[:, blk_a * D:])
        ident = singles.tile([P, P], f32)
        make_identity(nc, ident[:])
        for l in range(1, 3):
            nc.sync.dma_start(w_t[l][:], ws[l][:])
        br_t, gr_t, ber_t = [], [], []
        for l in range(3):
            br_t.append(singles.tile([P, D], f32, tag=f"br{l}",
                                     name=f"br{l}_t"))
            nc.sync.dma_start(br_t[l][:], brs[l][:])
        for l in range(2):
            gr_t.append(singles.tile([P, D], f32, tag=f"gr{l}",
                                     name=f"gr{l}_t"))
            nc.sync.dma_start(gr_t[l][:], grs[l][:])
            ber_t.append(singles.tile([P, D], f32, tag=f"ber{l}",
                                      name=f"ber{l}_t"))
            nc.sync.dma_start(ber_t[l][:], bers[l][:])
        ga_t = singles.tile([P, na16], i16)
        nc.sync.dma_start(ga_t[:], ga[:])
        gb_t = singles.tile([P, nb16], i16)
        nc.sync.dma_start(gb_t[:], gb[:])
        eps_t = singles.tile([P, 1], f32)
        nc.vector.memset(eps_t[:], eps)

        y_own_a = dram.tile([split, D], bf16)
        y_own_b = dram.tile([rows_b, D], bf16)
        tables_a = [dram.tile([split * n_cores, D], bf16,
                              addr_space="Shared", name=f"table_a{l}")
                    for l in range(n_layers)]
        tables_b = [dram.tile([rows_b * n_cores, D], bf16,
                              addr_space="Shared", name=f"table_b{l}")
                    for l in range(n_layers)]

        qn = [0]

        def gather(stage_tile, n_chunks, tab, gidx_t, col0):
            n_idx = n_chunks * P
            nc.gpsimd.dma_gather(
                out_ap=stage_tile[:, 0:n_chunks, :], in_ap=tab,
                idxs_ap=gidx_t[:, col0:col0 + n_idx // 16],
                num_idxs=n_idx, num_idxs_reg=n_idx, elem_size=D,
                single_packet=SINGLE_PACKET, queue_num=qn[0] % NQ)
            qn[0] += 1

        alu_add = Alu.add

        def z_blocks(layer, h_src, y_sb, b0, b1, transposed=False):
            """y[:, b0:b1 blocks] = dinv * (h_src @ W_layer), bf16.

            transposed=True: h_src is already feature-major (lhsT-ready).
            """
            for b in range(b0, b1):
                bs = slice(b * D, (b + 1) * D)
                if transposed:
                    hT_ap = h_src[:, bs]
                else:
                    tp = ps_t.tile([P, P], f32, tag="tp", name="tp")
                    nc.tensor.transpose(out=tp[:], in_=h_src[:, bs],
                                        identity=ident[:])
                    hT = htp.tile([P, P], f32, tag="hT", name="hT")
                    nc.scalar.copy(hT[:], tp[:])
                    hT_ap = hT[:]
                zp = ps_z.tile([P, D], f32, tag="zp", name="zp")
                nc.tensor.matmul(out=zp[:], lhsT=hT_ap, rhs=w_t[layer][:],
                                 start=True, stop=True)
                nc.scalar.activation(y_sb[:, bs], zp[:], Act.Copy,
                                     scale=dinv_t[:, b:b + 1])

        def fire_ag_a(y_sb, l):
            yv = y_own_a[:].rearrange("(p b) j -> p b j", b=blk_a)
            sv = y_sb[:, 0:blk_a * D].rearrange("p (b j) -> p b j", j=D)
            nc.sync.dma_start(yv, sv)
            nc.gpsimd.collective_compute(
                "AllGather", Alu.bypass,
                ins=[y_own_a[:].opt()], outs=[tables_a[l][:].opt()],
                replica_groups=[list(range(n_cores))])

        def fire_ag_b(y_sb, l):
            yv = y_own_b[:].rearrange("(p b) j -> p b j",
                                      b=n_blocks - blk_a)
            sv = y_sb[:, blk_a * D:].rearrange("p (b j) -> p b j", j=D)
            nc.sync.dma_start(yv, sv)
            nc.gpsimd.collective_compute(
                "AllGather", Alu.bypass,
                ins=[y_own_b[:].opt()], outs=[tables_b[l][:].opt()],
                replica_groups=[list(range(n_cores))])

        # layer-0 message table from the input features (feature-major)
        y_sb = ypool.tile([P, n_blocks * D], bf16, tag="y")
        z_blocks(0, xT, y_sb, 0, blk_a, transposed=True)
        fire_ag_a(y_sb, 0)
        z_blocks(0, xT, y_sb, blk_a, n_blocks, transposed=True)
        fire_ag_b(y_sb, 0)

        for layer in range(n_layers):
            W = D_OUT if layer == n_layers - 1 else D
            acc_sb = apool.tile([P, n_blocks * D], f32, tag="acc")

            g_tiles = {}
            s_tiles = {}

            def stage_for(flat_chunk, n_chunks_tot, tab, gidx_t, s_dram,
                          tag):
                g = flat_chunk // GQ
                if (tag, g) not in g_tiles:
                    n_in_g = min(GQ, n_chunks_tot - g * GQ)
                    t = stage.tile([P, GQ, D], bf16, tag="stg", name="stg")
                    gather(t, n_in_g, tab, gidx_t, g * GQ * P // 16)
                    g_tiles[(tag, g)] = t
                gs = flat_chunk // SGQ
                if (tag, gs) not in s_tiles:
                    n_in_s = min(SGQ, n_chunks_tot - gs * SGQ)
                    st = spool.tile([P, SGQ * P], bf16, tag="sst",
                                    name="sst")
                    nc.sync.dma_start(
                        st[:, 0:n_in_s * P],
                        s_dram[:, gs * SGQ * P:(gs * SGQ + n_in_s) * P])
                    s_tiles[(tag, gs)] = st
                msg = g_tiles[(tag, g)][:, flat_chunk % GQ, 0:W]
                ci = flat_chunk % SGQ
                s_ap = s_tiles[(tag, gs)][:, ci * P:(ci + 1) * P]
                return msg, s_ap

            # ---- phase A: table_a chunks -> acc_sb (= A_a@y + y) ----
            pend_a = []

            def flush_a():
                bb, aa = pend_a.pop(0)
                bbs = slice(bb * D, bb * D + W)
                nc.vector.tensor_tensor(out=acc_sb[:, bbs], in0=aa[:],
                                        in1=y_sb[:, bbs], op=alu_add)

            for b in range(n_blocks):
                acc = ps_a.tile([P, W], f32, tag="pacc")
                for k in range(ca[b]):
                    msg, s_ap = stage_for(aoff[b] + k, nch_a,
                                          tables_a[layer][:],
                                          ga_t, sa_in, "sta")
                    nc.tensor.matmul(out=acc[:], lhsT=s_ap, rhs=msg,
                                     start=(k == 0), stop=(k == ca[b] - 1))
                pend_a.append((b, acc))
                if len(pend_a) > 3:
                    flush_a()
            while pend_a:
                flush_a()

            # ---- phase B: table_b chunks + epilogue ----
            has_bias, has_g, has_be = flags[layer]
            if layer < 2:
                h_nxt = hpool.tile([P, n_blocks * D], f32, tag="h")
                # LayerNorm batch stats: mean/var per block, batched recip
                mvall = lnp.tile([P, 2 * n_blocks], f32, tag="mv")
                negmu = lnp.tile([P, 2 * n_blocks], f32, tag="nm")
                stdall = lnp.tile([P, n_blocks], f32, tag="sd")
                rall = lnp.tile([P, n_blocks], f32, tag="ra")
            pend_b = []

            def epilogue(b, acc):
                bs = slice(b * D, b * D + W)
                # v = acc_a + acc_b (+ y folded into acc_sb already)
                if layer < 2:
                    dst_ap = h_nxt[:, bs]
                else:
                    v_t = epi.tile([P, W], f32, tag="v", name="v_t")
                    dst_ap = v_t[:]
                nc.vector.tensor_tensor(out=dst_ap, in0=acc[:],
                                        in1=acc_sb[:, bs], op=alu_add)
                if has_bias:
                    # bias is per-feature: needs replicated tile + TT add
                    nc.scalar.activation(dst_ap, dst_ap, Act.Copy,
                                         scale=dinv_t[:, b:b + 1])
                    nc.vector.tensor_tensor(out=dst_ap, in0=dst_ap,
                                            in1=br_t[layer][:], op=alu_add)
                    if layer < 2:
                        nc.scalar.activation(dst_ap, dst_ap, Act.Relu)
                elif layer < 2:
                    nc.scalar.activation(dst_ap, dst_ap, Act.Relu,
                                         scale=dinv_t[:, b:b + 1])
                else:
                    nc.scalar.activation(dst_ap, dst_ap, Act.Copy,
                                         scale=dinv_t[:, b:b + 1])
                if layer < 2:
                    stats = small.tile([P, 6], f32, tag="st")
                    nc.vector.bn_stats(out=stats[:], in_=dst_ap)
                    nc.vector.bn_aggr(out=mvall[:, 2 * b:2 * b + 2],
                                      in_=stats[:])
                    nc.scalar.activation(stdall[:, b:b + 1],
                                         mvall[:, 2 * b + 1:2 * b + 2],
                                         Act.Sqrt, bias=eps_t[:])
                else:
                    nc.sync.dma_start(out_t[b * P:(b + 1) * P, :],
                                      dst_ap)

            def ln_apply(b0, b1):
                # batched: negmu = -mv (mean cols), rall = 1/stdall
                nc.vector.tensor_scalar(out=negmu[:, 2 * b0:2 * b1],
                                        in0=mvall[:, 2 * b0:2 * b1],
                                        scalar1=-1.0, scalar2=None,
                                        op0=Alu.mult)
                nc.vector.reciprocal(rall[:, b0:b1], stdall[:, b0:b1])
                for b in range(b0, b1):
                    bs = slice(b * D, (b + 1) * D)
                    nmr = small.tile([P, 1], f32, tag="nmr", name="nmr")
                    nc.vector.tensor_tensor(out=nmr[:],
                                            in0=negmu[:, 2 * b:2 * b + 1],
                                            in1=rall[:, b:b + 1],
                                            op=Alu.mult)
                    needs_post = has_g or has_be
                    if needs_post:
                        v_t = epi.tile([P, D], f32, tag="v", name="v_t")
                        apply_out = v_t[:]
                    else:
                        apply_out = h_nxt[:, bs]
                    nc.scalar.activation(apply_out, h_nxt[:, bs],
                                         Act.Identity, bias=nmr[:],
                                         scale=rall[:, b:b + 1])
                    if has_g and has_be:
                        nc.vector.tensor_tensor(out=apply_out, in0=apply_out,
                                                in1=gr_t[layer][:],
                                                op=Alu.mult)
                        nc.vector.tensor_tensor(out=h_nxt[:, bs],
                                                in0=apply_out,
                                                in1=ber_t[layer][:],
                                                op=alu_add)
                    elif has_g:
                        nc.vector.tensor_tensor(out=h_nxt[:, bs],
                                                in0=apply_out,
                                                in1=gr_t[layer][:],
                                                op=Alu.mult)
                    elif has_be:
                        nc.vector.tensor_tensor(out=h_nxt[:, bs],
                                                in0=apply_out,
                                                in1=ber_t[layer][:],
                                                op=alu_add)

            n_done = [0]
            y_nxt = None

            def pop_epilogue():
                nonlocal y_nxt
                epilogue(*pend_b.pop(0))
                n_done[0] += 1
                if n_done[0] == blk_a and layer < 2:
                    # first half of h_nxt finished: start next layer's
                    # message table + AllGather while phase B continues
                    ln_apply(0, blk_a)
                    y_nxt = ypool.tile([P, n_blocks * D], bf16, tag="y",
                                       name="y_nxt")
                    z_blocks(layer + 1, h_nxt, y_nxt, 0, blk_a)
                    fire_ag_a(y_nxt, layer + 1)

            for b in range(n_blocks):
                acc = ps_a.tile([P, W], f32, tag="pacc")
                for k in range(cb[b]):
                    msg, s_ap = stage_for(boff[b] + k, nch_b,
                                          tables_b[layer][:],
                                          gb_t, sb_in, "stb")
                    nc.tensor.matmul(out=acc[:], lhsT=s_ap, rhs=msg,
                                     start=(k == 0), stop=(k == cb[b] - 1))
                pend_b.append((b, acc))
                if len(pend_b) > 3:
                    pop_epilogue()
            while pend_b:
                pop_epilogue()

            if layer < 2:
                ln_apply(blk_a, n_blocks)
                z_blocks(layer + 1, h_nxt, y_nxt, blk_a, n_blocks)
                fire_ag_b(y_nxt, layer + 1)
                h_cur = h_nxt
                y_sb = y_nxt

    nc.compile()
    return nc


# ----------------------------------------------------------------------------
# Full-size entry point
# ----------------------------------------------------------------------------

N_NODES = 50000
N_CORES = 8
N_BLOCKS = 49            # 49*128 = 6272 nodes per core, 50176 padded
BLK_A = 17               # blocks per rank in table_a (8*17*128 = 17408 rows;
                         # table_b 8*32*128 = 32768 rows, max int16 idx 32767)

_KERNEL_CACHE = {}


def make_input_maps(x, edge_index, W1, b1, W2, b2, W3, b3, g1, be1, g2, be2,
                    n_nodes, n_cores, n_blocks, blk_a):
    x = np.asarray(x, np.float32)
    pre = preprocess(np.asarray(edge_index), n_nodes, n_cores, n_blocks,
                     blk_a)
    xsh = shard_x(x, n_nodes, n_cores, n_blocks)
    w3p = np.zeros((D, D), np.float32)
    w3 = np.asarray(W3, np.float32)
    w3p[:, :w3.shape[1]] = w3
    b3p = np.zeros((D,), np.float32)
    b3a = np.asarray(b3, np.float32)
    b3p[:b3a.shape[0]] = b3a
    rep = lambda a: np.ascontiguousarray(
        np.tile(np.asarray(a, np.float32)[None, :], (P, 1)))

    bs = [np.asarray(b1, np.float32), np.asarray(b2, np.float32), b3p]
    gs = [np.asarray(g1, np.float32), np.asarray(g2, np.float32)]
    bes = [np.asarray(be1, np.float32), np.asarray(be2, np.float32)]
    flags = tuple(
        (bool(np.any(bs[l] != 0.0)),
         bool(l < 2 and np.any(gs[l] != 1.0)),
         bool(l < 2 and np.any(bes[l] != 0.0)))
        for l in range(3))
    pre["flags"] = flags

    # pad wrapped idx arrays up to the group-aligned width the kernel expects
    nch_a = sum(pre["CA"])
    nch_b = sum(pre["CB"])
    na16 = ((nch_a + GQ - 1) // GQ * GQ) * P // 16
    nb16 = ((nch_b + GQ - 1) // GQ * GQ) * P // 16

    def padw(a, w):
        if a.shape[1] < w:
            a = np.concatenate(
                [a, np.zeros((P, w - a.shape[1]), np.int16)], axis=1)
        return np.ascontiguousarray(a)

    shared = {
        "w0": np.asarray(W1, np.float32), "w1": np.asarray(W2, np.float32),
        "w2": w3p,
        "br0": rep(bs[0]), "br1": rep(bs[1]), "br2": rep(bs[2]),
        "gr0": rep(gs[0]), "gr1": rep(gs[1]),
        "ber0": rep(bes[0]), "ber1": rep(bes[1]),
    }
    in_maps = []
    for c in range(n_cores):
        pc = pre["cores"][c]
        in_maps.append({
            "xs": xsh[c], "ga": padw(pc["ga"], na16),
            "gb": padw(pc["gb"], nb16),
            "sa": pc["sa"], "sb": pc["sb"],
            "dinvb": pc["dinvb"], **shared,
        })
    return in_maps, pre


def kernel(x, edge_index, W1, b1, W2, b2, W3, b3, g1, be1, g2, be2):
    from concourse.bass_utils import run_bass_kernel_spmd

    in_maps, pre = make_input_maps(
        x, edge_index, W1, b1, W2, b2, W3, b3, g1, be1, g2, be2,
        N_NODES, N_CORES, N_BLOCKS, BLK_A)
    key = (N_CORES, N_BLOCKS, BLK_A, pre["CA"], pre["CB"], pre["flags"])
    if key not in _KERNEL_CACHE:
        _KERNEL_CACHE[key] = build_kernel(N_CORES, N_BLOCKS, BLK_A,
                                          pre["CA"], pre["CB"],
                                          pre["flags"])
    nc = _KERNEL_CACHE[key]

    res = run_bass_kernel_spmd(nc, in_maps, core_ids=list(range(N_CORES)))
    out = np.concatenate([res.results[c]["out"] for c in range(N_CORES)],
                         axis=0)
    return out[:N_NODES]
